# revision 8
# baseline (speedup 1.0000x reference)
"""HadLinear TRN2 kernel: out = fwht_1024blocks(x)/sqrt(1024) @ W.T

Math: fwht on 1024-blocks is x @ H_bd with H_bd = blockdiag(H_1024 x4),
H_1024 = H_8 (x) H_128 (Sylvester/natural order, index k = j2*128 + j1).
Per core (row-shard of x):
  Prep:    x, W cast to bf16 DRAM scratch (contiguous loads, full DMA rate)
  Phase A: xh^T tiles = butterfly_8( H_128 @ x^T chunks ); x^T chunks come
           from bf16 XBAR dma-transpose loads (fp32 strided loads are ~20x
           slower: 4B-per-partition descriptors)
  Phase B: C tiles = (xh^T slice).T @ (W^T tile): lhsT = A k x m-sub slice,
           rhs = W^T k x n tile -> PSUM [m-sub, n] accumulated over k,
           stored contiguously to y.
The 1/sqrt(1024) = 2^-5 scale is folded into H_128 (exact in bf16).

Self-contained: hardcodes shapes B=4, S=4096, D_in=D_out=4096, 8 cores.
"""

import numpy as np
import ml_dtypes

import concourse.bacc as bacc
import concourse.mybir as mybir
import concourse.tile as tile
from concourse.bass_utils import run_bass_kernel_spmd

P = 128
N_CORES = 8
B_FULL, S_FULL, D = 4, 4096, 4096
M_FULL = B_FULL * S_FULL          # 16384 rows total
M_CORE = M_FULL // N_CORES        # 2048 rows per core
HAD = 1024                        # hadamard block
STRIP = 512                       # row-strip width (phase A moving free dim)
NSTRIP = 512                      # out-feature strip width


def _h128_np():
    """H_128 (natural order) scaled by 1/sqrt(1024) = 2^-5; exact in bf16."""
    h = np.array([[(-1.0) ** bin(i & j).count("1") for j in range(P)]
                  for i in range(P)])
    return (h / 32.0).astype(ml_dtypes.bfloat16)


def build_nc(m_core=M_CORE, d=D, n_out=D, halves=2):
    """Build the per-core Bass kernel (SPMD: same program all cores)."""
    f32, bf16 = mybir.dt.float32, mybir.dt.bfloat16
    nc = bacc.Bacc(None, target_bir_lowering=False, debug=False)

    xs = nc.declare_dram_parameter("xs", [m_core, d], f32, isOutput=False)
    w = nc.declare_dram_parameter("w", [n_out, d], f32, isOutput=False)
    h = nc.declare_dram_parameter("h", [P, P], bf16, isOutput=False)
    y = nc.declare_dram_parameter("y", [m_core, n_out], f32, isOutput=True)

    n_blk = d // HAD
    kt_total = d // P                 # 32 k-tiles
    ms_total = m_core // STRIP        # row strips (4 full-size)
    ms_per_half = ms_total // halves
    msub_per_half = (m_core // halves) // P   # 8 output-row tiles per half
    ns_total = n_out // NSTRIP        # 8 out strips

    with tile.TileContext(nc) as tc:
        with (
            tc.tile_pool(name="dram", bufs=1, space="DRAM") as dramp,
            tc.tile_pool(name="const", bufs=1) as constp,
            tc.tile_pool(name="prep", bufs=2) as prep,
            tc.tile_pool(name="apool", bufs=(kt_total * ms_per_half * 5) // 4) as apool,
            tc.tile_pool(name="xbf", bufs=9) as xbfp,
            tc.tile_pool(name="bfly", bufs=10) as bflyp,
            tc.tile_pool(name="wbf", bufs=kt_total + 8) as wbfp,
            tc.tile_pool(name="outp", bufs=4) as outp,
            tc.tile_pool(name="ps", bufs=8, space="PSUM") as psp,
        ):
            h128 = constp.tile([P, P], bf16, tag="h", name="h128")
            nc.sync.dma_start(out=h128[:], in_=h[:])

            # bf16 DRAM scratch, written once with contiguous full-rate DMA
            wbf_d = dramp.tile([n_out, d], bf16, name="wbf_d")
            xbf_d = dramp.tile([m_core, d], bf16, name="xbf_d")
            CW = 2048
            # x first: phase A depends only on xbf_d, so W-prep overlaps it
            for src_ap, dst_ap, rows, pfx in ((xs, xbf_d, m_core, "x"),
                                              (w, wbf_d, n_out, "w")):
                for i in range(rows // P):
                    for c0 in range(0, d, CW):
                        t = prep.tile([P, CW], f32, tag="prep32",
                                      name=f"{pfx}p_{i}_{c0}")
                        nc.sync.dma_start(
                            out=t[:], in_=src_ap[i * P:(i + 1) * P, c0:c0 + CW])
                        tb = prep.tile([P, CW], bf16, tag="prep16",
                                       name=f"{pfx}pb_{i}_{c0}")
                        nc.any.tensor_copy(out=tb[:], in_=t[:])
                        nc.sync.dma_start(
                            out=dst_ap[i * P:(i + 1) * P, c0:c0 + CW], in_=tb[:])

            for half in range(halves):
                a_tiles = {}
                # ---- Phase A: FWHT of this half's row strips ----
                for msl in range(ms_per_half):
                    ms = half * ms_per_half + msl
                    r0 = ms * STRIP
                    for blk in range(n_blk):
                        # butterfly stage 1 fused with PSUM eviction: DVE may
                        # read only ONE input from PSUM, so bounce V[j] to
                        # SBUF (c), then s1 = c +/- V[j+4] (one PSUM input).
                        s1 = [None] * 8
                        for j in range(4):
                            vv = [None, None]
                            for idx, jj in enumerate((j, j + 4)):
                                k0 = blk * HAD + jj * P
                                xbf = xbfp.tile([P, STRIP], bf16,
                                                tag="xbf", name=f"xbf_{ms}_{blk}_{jj}")
                                nc.sync.dma_start_transpose(
                                    xbf[:], xbf_d[r0:r0 + STRIP, k0:k0 + P])
                                vt = psp.tile([P, STRIP], f32,
                                              tag="ps", name=f"v_{ms}_{blk}_{jj}")
                                nc.tensor.matmul(vt[:], lhsT=h128[:], rhs=xbf[:],
                                                 start=True, stop=True)
                                vv[idx] = vt
                            cj = bflyp.tile([P, STRIP], f32, tag="bfly",
                                            name=f"c_{ms}_{blk}_{j}")
                            nc.any.tensor_copy(out=cj[:], in_=vv[0][:])
                            s1[j] = bflyp.tile([P, STRIP], f32, tag="bfly",
                                               name=f"s1_{ms}_{blk}_{j}")
                            s1[j + 4] = bflyp.tile([P, STRIP], f32, tag="bfly",
                                                   name=f"s1_{ms}_{blk}_{j+4}")
                            nc.vector.tensor_add(out=s1[j][:], in0=cj[:], in1=vv[1][:])
                            nc.vector.tensor_sub(out=s1[j + 4][:], in0=cj[:], in1=vv[1][:])
                        s2 = [None] * 8
                        for j in (0, 1, 4, 5):
                            s2[j] = bflyp.tile([P, STRIP], f32, tag="bfly",
                                               name=f"s2_{ms}_{blk}_{j}")
                            s2[j + 2] = bflyp.tile([P, STRIP], f32, tag="bfly",
                                                   name=f"s2_{ms}_{blk}_{j+2}")
                            nc.vector.tensor_add(out=s2[j][:], in0=s1[j][:], in1=s1[j + 2][:])
                            nc.vector.tensor_sub(out=s2[j + 2][:], in0=s1[j][:], in1=s1[j + 2][:])
                        for j in (0, 2, 4, 6):
                            kt_a, kt_b = blk * 8 + j, blk * 8 + j + 1
                            ta = apool.tile([P, STRIP], bf16, tag="A",
                                            name=f"A_{half}_{kt_a}_{msl}")
                            tb = apool.tile([P, STRIP], bf16, tag="A",
                                            name=f"A_{half}_{kt_b}_{msl}")
                            nc.vector.tensor_add(out=ta[:], in0=s2[j][:], in1=s2[j + 1][:])
                            nc.vector.tensor_sub(out=tb[:], in0=s2[j][:], in1=s2[j + 1][:])
                            a_tiles[(kt_a, msl)] = ta
                            a_tiles[(kt_b, msl)] = tb

                # ---- Phase B: C tiles = A-slice.T @ W^T tile, contiguous out ----
                for ns in range(ns_total):
                    n0 = ns * NSTRIP
                    wbf = [None] * kt_total
                    for kt in range(kt_total):
                        wt = wbfp.tile([P, NSTRIP], bf16, tag="wbf",
                                       name=f"wbf_{half}_{ns}_{kt}")
                        nc.sync.dma_start_transpose(
                            wt[:], wbf_d[n0:n0 + NSTRIP, kt * P:(kt + 1) * P])
                        wbf[kt] = wt
                    for gr in range(msub_per_half // 4):
                        subs = [gr * 4 + i for i in range(4)]
                        cps = {g: psp.tile([P, NSTRIP], f32, tag="ps",
                                           name=f"c_{half}_{ns}_{g}")
                               for g in subs}
                        for kt in range(kt_total):
                            for g in subs:
                                msl, sub = divmod(g, STRIP // P)
                                nc.tensor.matmul(
                                    cps[g][:],
                                    lhsT=a_tiles[(kt, msl)][:, sub * P:(sub + 1) * P],
                                    rhs=wbf[kt][:],
                                    start=(kt == 0), stop=(kt == kt_total - 1),
                                )
                        for g in subs:
                            r0 = (half * msub_per_half + g) * P
                            cout = outp.tile([P, NSTRIP], f32, tag="outp",
                                             name=f"co_{half}_{ns}_{g}")
                            nc.any.tensor_copy(out=cout[:], in_=cps[g][:])
                            nc.sync.dma_start(out=y[r0:r0 + P, n0:n0 + NSTRIP],
                                              in_=cout[:])
    nc.compile()
    return nc


_CACHE = {}


def _get_nc():
    if "nc" not in _CACHE:
        _CACHE["nc"] = build_nc()
    return _CACHE["nc"]


def run(x, weight, trace=False):
    assert x.shape == (B_FULL, S_FULL, D) and weight.shape == (D, D)
    nc = _get_nc()
    xf = np.ascontiguousarray(np.asarray(x, dtype=np.float32).reshape(M_FULL, D))
    wf = np.ascontiguousarray(np.asarray(weight, dtype=np.float32))
    hh = _h128_np()
    in_maps = [
        {"xs": xf[c * M_CORE:(c + 1) * M_CORE], "w": wf, "h": hh}
        for c in range(N_CORES)
    ]
    res = run_bass_kernel_spmd(nc, in_maps, core_ids=list(range(N_CORES)),
                               trace=trace)
    yv = np.concatenate([r["y"] for r in res.results], axis=0)
    return yv.reshape(B_FULL, S_FULL, D), res


def kernel(x, weight):
    return run(x, weight)[0]
